# revision 4
# baseline (speedup 1.0000x reference)
"""Trainium2 Bass kernel for nn_Attention_45148696216907 — bf16 rewrite.

Math (per batch b, head h; no softmax -> matmul associativity applies):
    q  = x @ Wq.T + bq                  (n, H*D)
    k  = context @ Wk.T + bk            (n, D)     kv_heads = 1
    v  = context @ Wv.T + bv            (n, D)
    qn = l2norm(q per head), kn = l2norm(k)
    out_h = scale * qn_h @ (kn.T @ v)   (n, D)     <- (q@kT)@v == q@(kT@v)

Sharding: 4-way data parallel over batch x 2-way tensor parallel over heads.
All matmuls in bf16 (rel err ~4e-3 vs the 2e-2 gate), halving HBM traffic
vs f32r and keeping the PE at 1 cycle/row for every matmul shape.

Per-core dataflow (all device operands pre-transposed host-side):
    k/v:   pk[t,256] = sum_a ctT[a][:,t128].T @ wkvT[a]     (N=256)
    kn:    rsqrt(sum_d k^2) via ACT square+accum, DVE scalar-mul
    kv:    pkv[d,dv] += kn.T @ v        (one PSUM group, all token blocks)
    qT:    pq[j,t] = sum_a wqT[a,j].T @ xT[a,t512]          (N=512)
    sumsq: ones.T @ (q+bq)^2 -> replicated over partitions  (N=512)
    outT:  po[dv,t] = (scale*kv).T @ qT; o = po * rsqrt(sumsq)

Emission is software-pipelined via a pending-closure queue: the sumsq/outT
matmuls of head j and the kv matmul of token block t are emitted after the
NEXT big matmul group, so the PE never waits on the ACT/DVE chains and the
HAM clock stays at 2.4 GHz.  kv token-block units are interleaved between
q chunks to keep the ct/xt/wq DMA streams inside the HBM bandwidth budget;
outT for the first two q chunks is deferred (q pre-normalized) until kv is
complete and drained 2-per-head-slot inside chunks 2-3.

DMA rings: SP(sync) carries biases + ct units + output stores; ACT(scalar)
carries wkv/wq/xt (all issued dependency-free at t=0 to avoid blocking ACT
compute behind a waiting trigger).
"""

import sys

sys.path.insert(0, "/opt/trn_rl_repo")

import numpy as np
import ml_dtypes

import concourse.bass as bass
import concourse.mybir as mybir
import concourse.tile as tile
from concourse import bacc
from concourse.bass_utils import run_bass_kernel_spmd

F32 = mybir.dt.float32
BF16 = mybir.dt.bfloat16
AF = mybir.ActivationFunctionType
MUL = mybir.AluOpType.mult

B, N, DIM = 4, 2048, 2048
HEADS, D = 16, 128
N_CORES = 8
HGRP = 2                  # head-group shards
HL = HEADS // HGRP        # heads per core = 8
JW = HL * D               # q feature width per core = 1024
SCALE = 1.0 / np.sqrt(np.float32(D))
KB = DIM // 128           # 16 contraction blocks
NTB = N // 128            # 16 token blocks
MC = N // 512             # 4 q chunks


def _ap(t, offset, pattern):
    return bass.AP(tensor=t.tensor, offset=t.offset + offset, ap=pattern)


def _emit_body(ctx, nc: bass.Bass, tc: tile.TileContext, io, split=False,
               cc=True):
    xt, ct, wq, wkv, bq, bkv, o = io
    ctw = N // 2 if split else N          # ct token width in DRAM
    n_units = 4 if split else 8
    kv_last_tb = n_units * 2 - 1

    consts = ctx.enter_context(tc.tile_pool(name="consts", bufs=1))
    ctp = ctx.enter_context(tc.tile_pool(name="ctp", bufs=4))
    xtp = ctx.enter_context(tc.tile_pool(name="xtp", bufs=4))
    knvp = ctx.enter_context(tc.tile_pool(name="knvp", bufs=4))
    qtp = ctx.enter_context(tc.tile_pool(name="qtp", bufs=20))
    q2p = ctx.enter_context(tc.tile_pool(name="q2p", bufs=3))
    rnp = ctx.enter_context(tc.tile_pool(name="rnp", bufs=3))
    outp = ctx.enter_context(tc.tile_pool(name="outp", bufs=4))
    stats = ctx.enter_context(tc.tile_pool(name="stats", bufs=3))
    psum = ctx.enter_context(tc.tile_pool(name="psum", bufs=2, space="PSUM"))

    # ---- constants + input DMA issue (order = ring order) ------------------
    # DMA data phases serialize per HWDGE ring, and a trigger occupies the
    # issuing engine's queue for the transfer - so NOTHING DMA-heavy may sit
    # on the ACT queue ahead of ACT compute.  SP (no compute) carries every
    # startup-critical load in consumption order; the idle gpsimd SWDGE ring
    # carries the late xt chunks; output stores join the SP ring at the end.
    bkv_bc = consts.tile([128, 256], F32)
    bq_sb = consts.tile([128, HL], F32)
    wkv_sb = consts.tile([128, KB, 256], BF16)
    wq_sb = consts.tile([128, KB, JW], BF16)
    xts = [xtp.tile([128, KB, 512], BF16, name=f"xt_{mc}", tag="xt")
           for mc in range(MC)]

    def load_wkv(h):
        nc.sync.dma_start(
            out=wkv_sb[:, h * 8:(h + 1) * 8, :],
            in_=_ap(wkv, h * 8 * 128 * 256, [[256, 128], [128 * 256, 8], [1, 256]]))

    def load_wq_pair(p):
        nc.sync.dma_start(
            out=wq_sb[:, :, p * 256:(p + 1) * 256],
            in_=_ap(wq, p * 256, [[JW, 128], [128 * JW, KB], [1, 256]]))

    def load_xt(mc, a0, na, eng):
        return eng.dma_start(
            out=xts[mc][:, a0:a0 + na, :],
            in_=_ap(xt, a0 * 128 * N + mc * 512, [[N, 128], [128 * N, na], [1, 512]]))

    ct_tiles = {}

    def issue_ct(u):
        ct_t = ctp.tile([128, KB, 256], BF16, name=f"ct_{u}", tag="ct")
        d = nc.sync.dma_start(
            out=ct_t,
            in_=_ap(ct, u * 256, [[ctw, 128], [128 * ctw, KB], [1, 256]]))
        ct_tiles[u] = ct_t
        return d

    # interleave the first wkv/ct0 half-loads so the cold PE starts ~3us in,
    # and push the (tiny, later-needed) bias loads behind them
    ct0_t = ctp.tile([128, KB, 256], BF16, name="ct_0", tag="ct")
    ct_tiles[0] = ct0_t
    load_wkv(0)
    nc.sync.dma_start(out=ct0_t[:, 0:8, :],
                      in_=_ap(ct, 0, [[ctw, 128], [128 * ctw, 8], [1, 256]]))
    load_wkv(1)
    ct0b_dma = nc.sync.dma_start(
        out=ct0_t[:, 8:16, :],
        in_=_ap(ct, 8 * 128 * ctw, [[ctw, 128], [128 * ctw, 8], [1, 256]]))
    nc.sync.dma_start(out=bkv_bc,
                      in_=bass.AP(tensor=bkv.tensor, offset=bkv.offset,
                                  ap=[[0, 128], [1, 256]]))
    nc.sync.dma_start(out=bq_sb, in_=bq.rearrange("(c p) -> p c", p=128))
    # xt0 rides the SWDGE ring, held behind ct0 so its data phase does not
    # steal HBM bandwidth from the SP ring's startup-critical stream
    from concourse.tile import add_dep_helper
    xt0a_dma = nc.gpsimd.dma_start(
        out=xts[0][:, 0:8, :],
        in_=_ap(xt, 0, [[N, 128], [128 * N, 8], [1, 512]]))
    add_dep_helper(xt0a_dma.ins, ct0b_dma.ins, sync=True,
                   reason="hold xt0 data phase behind ct0 startup stream")
    nc.gpsimd.dma_start(
        out=xts[0][:, 8:16, :],
        in_=_ap(xt, 8 * 128 * N, [[N, 128], [128 * N, 8], [1, 512]]))

    def load_q_inputs(units):
        """SP-ring FIFO order ~= consumption order; xt chunks ride SWDGE."""
        load_wq_pair(0)
        if units and len(units) > 0:
            issue_ct(units[0])
        load_wq_pair(1)
        if len(units) > 1:
            issue_ct(units[1])
        load_wq_pair(2)
        if len(units) > 2:
            issue_ct(units[2])
        load_wq_pair(3)
        ct_dmas = {}
        for u in units[3:]:
            ct_dmas[u] = issue_ct(u)
        # flatten the HBM demand curve: each late xt chunk's SWDGE data
        # phase is held behind a ct unit it would otherwise contend with
        # (xt_k is not consumed until long after that ct lands)
        from concourse.tile import add_dep_helper
        holds = {1: units[3] if len(units) > 3 else None,
                 2: units[5] if len(units) > 5 else None,
                 3: units[-1] if len(units) > 3 else None}
        for k in (1, 2, 3):
            d = load_xt(k, 0, 16, nc.gpsimd)
            u = holds.get(k)
            if u is not None and u in ct_dmas:
                add_dep_helper(d.ins, ct_dmas[u].ins, sync=True,
                               reason=f"hold xt{k} behind ct{u} stream")

    ones_f = consts.tile([128, 128], F32)
    nc.vector.memset(ones_f, 1.0)
    ones_sb = consts.tile([128, 128], BF16)
    nc.vector.tensor_copy(out=ones_sb, in_=ones_f)
    eps_sb = consts.tile([128, 1], F32)
    nc.vector.memset(eps_sb, 1e-30)
    kdump = consts.tile([128, 128], BF16)    # ACT square scratch (never read)

    pkv = psum.tile([128, 128], F32, name="pkv", tag="pkv", bufs=1)

    # ---- software-pipeline machinery --------------------------------------
    pend = []        # FIFO of closures emitting deferred PE work + chains
    deferred = []    # (mc, jb, qn) outT jobs waiting for kv
    state = {"kv": None}
    toggle = [0]

    def emit_dot(mc, jb, qn):
        po = psum.tile([128, 512], F32, name=f"dpo_{mc}_{jb}", tag="kvp",
                       bufs=2)
        nc.tensor.matmul(out=po, lhsT=state["kv"], rhs=qn, start=True,
                         stop=True)
        o_sb = outp.tile([128, 512], BF16, name=f"do_{mc}_{jb}", tag="o")
        if toggle[0] % 2 == 0:
            nc.scalar.copy(out=o_sb, in_=po)
        else:
            nc.vector.tensor_copy(out=o_sb, in_=po)
        toggle[0] += 1
        nc.sync.dma_start(out=o[jb, :, mc * 512:(mc + 1) * 512], in_=o_sb)

    def flush(ndef=0):
        while pend:
            pend.pop(0)()
        for _ in range(ndef):
            if state["kv"] is not None and deferred:
                emit_dot(*deferred.pop(0))

    # ---- k/v projection + kv accumulation (per 256-token unit) ------------
    def kv_unit(u):
        if u not in ct_tiles:
            issue_ct(u)
        ct_t = ct_tiles[u]
        for tt in range(2):
            tb = u * 2 + tt
            pk = psum.tile([128, 256], F32, name=f"pk_{tb}", tag="kvp",
                           bufs=2)
            for a in range(KB):
                nc.tensor.matmul(out=pk,
                                 lhsT=ct_t[:, a, tt * 128:(tt + 1) * 128],
                                 rhs=wkv_sb[:, a, :],
                                 start=(a == 0), stop=(a == KB - 1))
            flush()
            knv = knvp.tile([128, 256], BF16, name=f"knv_{tb}", tag="knv")
            nc.vector.tensor_add(out=knv, in0=pk, in1=bkv_bc)
            ks = stats.tile([128, 1], F32, name=f"ks_{tb}", tag="ks")
            nc.scalar.activation(out=kdump, in_=knv[:, 0:128],
                                 func=AF.Square, accum_out=ks)
            ksq = stats.tile([128, 1], F32, name=f"ksq_{tb}", tag="ksq")
            nc.scalar.activation(out=ksq, in_=ks, func=AF.Sqrt, bias=eps_sb)
            rk = stats.tile([128, 1], F32, name=f"rk_{tb}", tag="rk")
            nc.vector.reciprocal(out=rk, in_=ksq)
            nc.vector.tensor_scalar_mul(out=knv[:, 0:128], in0=knv[:, 0:128],
                                        scalar1=rk)

            def mk_kv(tb=tb, knv=knv):
                nc.tensor.matmul(out=pkv, lhsT=knv[:, 0:128],
                                 rhs=knv[:, 128:256],
                                 start=(tb == 0), stop=(tb == kv_last_tb))
            pend.append(mk_kv)

    # ---- kv finalization ---------------------------------------------------
    def emit_fin():
        def f():
            kv_sb = consts.tile([128, 128], BF16, name="kv_sb")
            nc.scalar.mul(out=kv_sb, in_=pkv, mul=float(SCALE))
            state["kv"] = kv_sb
        return f

    def emit_fin_split():
        def f():
            kv_part = consts.tile([128, 128], F32, name="kv_part")
            nc.scalar.copy(out=kv_part, in_=pkv)
            kv_in = nc.dram_tensor("kv_in", [128, 128], F32)
            kv_out = nc.dram_tensor("kv_out", [128, 128], F32)
            nc.gpsimd.dma_start(out=kv_in[:, :], in_=kv_part)
            if cc:
                nc.gpsimd.collective_compute(
                    "AllReduce", mybir.AluOpType.add,
                    replica_groups=[[2 * i, 2 * i + 1]
                                    for i in range(N_CORES // 2)],
                    ins=[kv_in[:, :]], outs=[kv_out[:, :]])
            else:
                nc.gpsimd.dma_start(out=kv_out[:, :], in_=kv_in[:, :])
            kv_red = consts.tile([128, 128], F32, name="kv_red")
            nc.gpsimd.dma_start(out=kv_red, in_=kv_out[:, :])
            kv_sb = consts.tile([128, 128], BF16, name="kv_sb")
            nc.scalar.mul(out=kv_sb, in_=kv_red, mul=float(SCALE))
            state["kv"] = kv_sb
        return f

    # ---- q projection chunk (8 heads x 512 tokens) -------------------------
    def mk_qtail(mc, jb, q2, qt, defer):
        def f():
            ps = psum.tile([128, 512], F32, name=f"ps_{mc}_{jb}", tag="big",
                           bufs=3)
            nc.tensor.matmul(out=ps, lhsT=ones_sb, rhs=q2, start=True,
                             stop=True)
            sq = rnp.tile([128, 512], F32, name=f"sq_{mc}_{jb}", tag="sq")
            nc.scalar.activation(out=sq, in_=ps, func=AF.Sqrt, bias=eps_sb)
            rn = rnp.tile([128, 512], F32, name=f"rn_{mc}_{jb}", tag="rn")
            nc.vector.reciprocal(out=rn, in_=sq)
            if state["kv"] is not None and not defer:
                po = psum.tile([128, 512], F32, name=f"po_{mc}_{jb}",
                               tag="big", bufs=3)
                nc.tensor.matmul(out=po, lhsT=state["kv"], rhs=qt,
                                 start=True, stop=True)
                o_sb = outp.tile([128, 512], BF16, name=f"o_{mc}_{jb}",
                                 tag="o")
                nc.vector.tensor_tensor(out=o_sb, in0=po, in1=rn, op=MUL)
                nc.sync.dma_start(out=o[jb, :, mc * 512:(mc + 1) * 512],
                                  in_=o_sb)
            else:
                nc.vector.tensor_tensor(out=qt, in0=qt, in1=rn, op=MUL)
                deferred.append((mc, jb, qt))
        return f

    def q_chunk(mc, ndef, defer, units_at=None):
        xt_t = xts[mc]
        for jb in range(HL):
            pq = psum.tile([128, 512], F32, name=f"pq_{mc}_{jb}", tag="qp",
                           bufs=2)
            for a in range(KB):
                nc.tensor.matmul(out=pq,
                                 lhsT=wq_sb[:, a, jb * 128:(jb + 1) * 128],
                                 rhs=xt_t[:, a, :],
                                 start=(a == 0), stop=(a == KB - 1))
            flush(ndef)
            if units_at and jb in units_at:
                kv_unit(units_at[jb])
            qt = qtp.tile([128, 512], BF16, name=f"qt_{mc}_{jb}", tag="qt")
            nc.vector.tensor_scalar_add(out=qt, in0=pq,
                                        scalar1=bq_sb[:, jb:jb + 1])
            q2 = q2p.tile([128, 512], BF16, name=f"q2_{mc}_{jb}", tag="q2")
            nc.scalar.activation(out=q2, in_=pq, func=AF.Square,
                                 bias=bq_sb[:, jb:jb + 1])
            pend.append(mk_qtail(mc, jb, q2, qt, defer))

    # ---- main sequence -----------------------------------------------------
    # kv units 0-1 feed the cold PE while wq/xt0 stream; the remaining units
    # are spread into q-chunk head slots so the ct stream never needs more
    # than ~20% of HBM bandwidth at once.  outT for chunks 0-1 is deferred
    # (q pre-normalized) and drained 2-per-head-slot in chunks 2-3.
    if split:
        kv_unit(0); kv_unit(1)
        load_q_inputs([2, 3])
        q_chunk(0, 0, defer=True, units_at={3: 2, 6: 3})
        pend.append(emit_fin_split())
        q_chunk(1, 0, defer=True)
        q_chunk(2, 2, defer=False)
        q_chunk(3, 2, defer=False)
    else:
        kv_unit(0); kv_unit(1)
        load_q_inputs([2, 3, 4, 5, 6, 7])
        q_chunk(0, 0, defer=True, units_at={2: 2, 4: 3, 6: 4})
        q_chunk(1, 0, defer=True, units_at={1: 5, 3: 6, 5: 7})
        pend.append(emit_fin())
        q_chunk(2, 2, defer=False)
        q_chunk(3, 2, defer=False)
    flush(99)


def build_nc(loop_k=1, split=False, cc=True):
    nc = bacc.Bacc(None)
    ctw = N // 2 if split else N
    xt = nc.declare_dram_parameter("xt", [DIM, N], BF16, isOutput=False)
    ct = nc.declare_dram_parameter("ct", [DIM, ctw], BF16, isOutput=False)
    wq = nc.declare_dram_parameter("wq", [DIM, JW], BF16, isOutput=False)
    wkv = nc.declare_dram_parameter("wkv", [DIM, 256], BF16, isOutput=False)
    bq = nc.declare_dram_parameter("bq", [JW], F32, isOutput=False)
    bkv = nc.declare_dram_parameter("bkv", [256], F32, isOutput=False)
    o = nc.declare_dram_parameter("o", [HL, D, N], BF16, isOutput=True)
    from contextlib import ExitStack
    with tile.TileContext(nc) as tc, ExitStack() as ctx:
        io = (xt[:, :], ct[:, :], wq[:, :], wkv[:, :], bq[:], bkv[:],
              o[:, :, :])
        if loop_k > 1:
            with tc.For_i(0, loop_k, 1,
                          hint_engines=(mybir.EngineType.PE,)):
                _emit_body(ctx, nc, tc, io, split=split, cc=cc)
        else:
            _emit_body(ctx, nc, tc, io, split=split, cc=cc)
    nc.compile()
    return nc


def make_in_maps(x, context, Wq, bq, Wk, bk, Wv, bv, split=False):
    bf = ml_dtypes.bfloat16
    x = np.asarray(x, np.float32)
    context = np.asarray(context, np.float32)
    Wq = np.asarray(Wq, np.float32)
    bq = np.asarray(bq, np.float32)
    bkv = np.ascontiguousarray(np.concatenate(
        [np.asarray(bk, np.float32), np.asarray(bv, np.float32)]))
    wkvt = np.ascontiguousarray(np.concatenate(
        [np.asarray(Wk, np.float32), np.asarray(Wv, np.float32)], 0).T.astype(bf))
    xts = [np.ascontiguousarray(x[b].T.astype(bf)) for b in range(B)]
    cts = [context[b].T.astype(bf) for b in range(B)]
    half = N // 2
    wqts = [np.ascontiguousarray(Wq[g * JW:(g + 1) * JW].T.astype(bf))
            for g in range(HGRP)]
    bqs = [np.ascontiguousarray(bq[g * JW:(g + 1) * JW]) for g in range(HGRP)]
    in_maps = []
    for c in range(N_CORES):
        bi, hg = c // HGRP, c % HGRP
        if split:
            ctm = np.ascontiguousarray(cts[bi][:, hg * half:(hg + 1) * half])
        else:
            ctm = np.ascontiguousarray(cts[bi])
        in_maps.append({
            "xt": xts[bi], "ct": ctm, "wq": wqts[hg], "wkv": wkvt,
            "bq": bqs[hg], "bkv": bkv,
        })
    return in_maps


def gather(results):
    out = np.empty((B, HEADS, N, D), dtype=np.float32)
    for c in range(N_CORES):
        bi, hg = c // HGRP, c % HGRP
        oc = np.asarray(results[c]["o"]).astype(np.float32)   # (HL, D, N)
        out[bi, hg * HL:(hg + 1) * HL] = oc.transpose(0, 2, 1)
    return out


_NC = None
_NC_SPLIT = False


def kernel(x, context, Wq, bq, Wk, bk, Wv, bv):
    """Full-input entry point: shard across 8 NeuronCores, run, gather."""
    global _NC, _NC_SPLIT
    args = (x, context, Wq, bq, Wk, bk, Wv, bv)
    if _NC is None:
        _NC, _NC_SPLIT = build_nc(split=False), False
    last_err = None
    for attempt in range(3):
        try:
            in_maps = make_in_maps(*args, split=_NC_SPLIT)
            res = run_bass_kernel_spmd(_NC, in_maps,
                                       core_ids=list(range(N_CORES)))
            return gather(res.results)
        except Exception as e:  # transient axon/NRT flakes
            last_err = e
    raise last_err
